# revision 19
# baseline (speedup 1.0000x reference)
"""nn_Attention4DDownsample — Trainium2 Bass/Tile kernel.

Sharding: pure data parallel over batch (8 cores x 32 examples), weights
replicated.  All matmuls bf16.  Host pre-folds BN into conv weights, fuses
the depthwise 3x3 s2 convs with their neighbouring 1x1 convs into 9 per-tap
dense matrices (pool folded into the centre tap), builds an im2col copy of
x for the taps, and pre-gathers the attention bias table.

Attention is computed transposed (keys on partitions):
  logitsT[key,(h,q)] = contraction of k (natural layout) with blockdiag(q)
so softmax exp/sums and the AV matmul need no on-chip transposes at all.
The softmax denominator is produced replicated across partitions via an
all-ones matmul, so normalisation is a plain elementwise multiply; AV fuses
head pairs into M=128 matmuls whose off-head quadrants are never read.
"""

import contextlib
import os
import sys

import numpy as np

for _p in ("/opt/trn_rl_repo", "/root/.axon_site/_ro/trn_rl_repo"):
    if _p not in sys.path and os.path.isdir(_p):
        sys.path.append(_p)

import ml_dtypes  # noqa: E402

import concourse.bacc as bacc  # noqa: E402
import concourse.mybir as mybir  # noqa: E402
import concourse.tile as tile  # noqa: E402

BF16 = mybir.dt.bfloat16
F32 = mybir.dt.float32
AF = mybir.ActivationFunctionType

B, DIM, RES = 256, 384, 14
H, KD, D = 8, 16, 64
NH_KD, DH = H * KD, H * D  # 128, 512
OUT_DIM = 768
RES2 = 7
N, N2 = RES * RES, RES2 * RES2  # 196, 49
SCALE = KD ** -0.5
NCORES = 8
E = B // NCORES  # 32 examples per core

KC = DIM // 128        # 3 contraction chunks of x-channels
VMC = DH // 128        # 4 m-chunks of v channels
PMC = OUT_DIM // 128   # 6 m-chunks of proj out channels
HQ = H * N2            # 392
# key split along the 14x14 grid: first 9 rows (126 keys) + last 5 (70 keys)
KEY0, KEY1 = 9 * RES, 5 * RES  # 126, 70
PAD = RES + 1  # 15 (top/left zero pad only)

NEG = 8   # examples per dwconv N-chunk (N = 8*49 = 392)
GEX = 4   # examples per attention group


def _bf(x):
    return np.ascontiguousarray(np.asarray(x, np.float32).astype(ml_dtypes.bfloat16))


def _f32(x):
    return np.ascontiguousarray(np.asarray(x, np.float32))


def host_prep(inputs):
    """Fold BN, fuse dwconvs, build im2col, gather bias."""
    ii = {k: (np.asarray(v) if np.asarray(v).dtype == np.int32
              else np.asarray(v, np.float32)) for k, v in inputs.items()}

    kw = ii["k_w"][:, :, 0, 0] * ii["k_bn_s"][:, None]          # [128,384]
    kb = ii["k_b"] * ii["k_bn_s"] + ii["k_bn_b"]                # [128]
    qw = SCALE * ii["q_bn_s"][:, None] * ii["q_proj_w"][:, :, 0, 0]   # [128,384]
    qlw = ii["q_local_w"][:, 0].reshape(DIM, 9).copy()                # [384,9]
    qlw[:, 4] += 1.0                                                  # pool
    q_taps = np.einsum("md,dt->tdm", qw, qlw)                         # [9,384,128]
    qb = (SCALE * (ii["q_bn_s"] * ii["q_proj_b"] + ii["q_bn_b"])
          + qw @ ii["q_local_b"])                                     # [128]
    vw = ii["v_w"][:, :, 0, 0] * ii["v_bn_s"][:, None]          # [512,384]
    vb = ii["v_b"] * ii["v_bn_s"] + ii["v_bn_b"]                # [512]
    vlw = ii["vl_w"][:, 0].reshape(DH, 9) * ii["vl_bn_s"][:, None]  # [512,9]
    vlb = ii["vl_b"] * ii["vl_bn_s"] + ii["vl_bn_b"]            # [512]
    w_taps = np.einsum("cd,ct->tdc", vw, vlw)                   # [9,384,512]
    # dwconv bias: vb contributes only where a tap hits the interior
    interior = np.zeros((9, N2), np.float32)
    for t in range(9):
        di, dj = t // 3, t % 3
        for i in range(RES2):
            for j in range(RES2):
                r, c = 2 * i + di - 1, 2 * j + dj - 1
                if 0 <= r < RES and 0 <= c < RES:
                    interior[t, i * RES2 + j] = 1.0
    blb = vlb[:, None] + vb[:, None] * (vlw @ interior)         # [512,49]
    blb_rep = np.tile(blb[:, None, :], (1, NEG, 1))             # [512,NEG,49]
    bias = ii["attn_biases"][:, ii["bias_idxs"]]                # [8,49,196]
    biasT = np.transpose(bias, (2, 0, 1)).reshape(N, HQ)        # [196,392]
    pw = ii["proj_w"][:, :, 0, 0] * ii["proj_bn_s"][:, None]    # [768,512]
    pb = ii["proj_b"] * ii["proj_bn_s"] + ii["proj_bn_b"]       # [768]

    shared = {
        "kw_t": _bf(kw.T),                        # [384,128]
        "kb": _f32(kb)[:, None],                  # [128,1]
        "q_taps": _bf(q_taps),                    # [9,384,128]
        "qb": _f32(qb)[:, None],                  # [128,1]
        "w_taps": _bf(w_taps),                    # [9,384,512]
        "vw_t": _bf(vw.T),                        # [384,512]
        "vb_rep": _f32(np.tile(vb[None, :], (128, 1))),  # [128,512]
        "blb_rep": _bf(blb_rep.reshape(DH, NEG * N2)),   # [512,392]
        "biasT": _bf(biasT),                      # [196,392]
        "pw_t": _bf(pw.T),                        # [512,768]
        "pb": _f32(np.ascontiguousarray(pb.reshape(PMC, 128).T)),  # [128,6]
        "ident": _bf(np.eye(128)),                # [128,128]
        "ones_t": _bf(np.ones((128, 128))),       # [128,128]
    }

    # x: shard; flat channel-major copy + host im2col of the 9 dw taps
    x = ii["x"].reshape(NCORES, E, DIM, RES, RES)
    xp = np.zeros((NCORES, E, DIM, PAD, PAD), np.float32)
    xp[:, :, :, 1:, 1:] = x
    xp = np.transpose(xp, (0, 2, 1, 3, 4))        # [8,384,E,15,15]
    x_shards = []
    for c in range(NCORES):
        xflat = _bf(np.transpose(x[c], (1, 0, 2, 3)).reshape(DIM, E * N))
        taps = np.stack(
            [xp[c][:, :, t // 3:t // 3 + 13:2, t % 3:t % 3 + 13:2]
             .reshape(DIM, E, N2) for t in range(9)], axis=1)  # [384,9,E,49]
        x_shards.append({"xfl": xflat, "xcol": _bf(taps.reshape(DIM, 9 * E * N2))})
    return shared, x_shards


def build_nc(e=E, loops=0, parts=15):
    """Build the Bass program for one core with `e` examples.

    loops>0 wraps the body in a hardware For loop (benchmark builds).
    parts: bitmask 1=dwconv 2=k/vT 4=attention 8=gelu/proj (bench only).
    """
    ndw = e // NEG      # dwconv / stream blocks
    ngrp = e // GEX     # attention groups
    assert e % NEG == 0 and e % GEX == 0

    nc = bacc.Bacc("TRN2", target_bir_lowering=False, debug=False,
                   enable_asserts=False, num_devices=NCORES)

    def din(name, shape, dtype=BF16):
        return nc.dram_tensor(name, list(shape), dtype, kind="ExternalInput").ap()

    dd = {
        "xfl_d": din("xfl", (DIM, e * N)),
        "xcol_d": din("xcol", (DIM, 9 * e * N2)),
        "kw_d": din("kw_t", (DIM, 128)),
        "kb_d": din("kb", (128, 1), F32),
        "qt_d": din("q_taps", (9, DIM, 128)),
        "qb_d": din("qb", (128, 1), F32),
        "wt_d": din("w_taps", (9, DIM, DH)),
        "vw_d": din("vw_t", (DIM, DH)),
        "vbr_d": din("vb_rep", (128, DH), F32),
        "blb_d": din("blb_rep", (DH, NEG * N2)),
        "bt_d": din("biasT", (N, HQ)),
        "pw_d": din("pw_t", (DH, OUT_DIM)),
        "pb_d": din("pb", (128, PMC), F32),
        "id_d": din("ident", (128, 128)),
        "on_d": din("ones_t", (128, 128)),
        "out_d": nc.dram_tensor("out", [e, OUT_DIM, N2], F32,
                                kind="ExternalOutput").ap(),
    }

    with tile.TileContext(nc) as tc:
        with (tc.For_i(0, loops, 1) if loops else contextlib.nullcontext()):
            build_body(nc, tc, e, ndw, ngrp, dd, parts)

    nc.compile()
    return nc


def build_body(nc, tc, e, ndw, ngrp, dd, parts=15):
    with tc.tile_pool(name="const", bufs=1) as cp:
        kw_sb = cp.tile([128, KC, 128], BF16, tag="kw")
        kb_sb = cp.tile([128, 1], F32, tag="kb")
        qb_sb = cp.tile([128, 1], F32, tag="qb")
        vbr_sb = cp.tile([128, DH], F32, tag="vbr")
        blb_sb = cp.tile([128, VMC, NEG * N2], BF16, tag="blb")
        bt_sb = cp.tile([128, 2, HQ], BF16, tag="bt")
        pw_sb = cp.tile([128, VMC, OUT_DIM], BF16, tag="pw")
        pb_sb = cp.tile([128, PMC], F32, tag="pb")
        id_sb = cp.tile([128, 128], BF16, tag="id")
        on_sb = cp.tile([128, 128], BF16, tag="on")
        vw_sb = cp.tile([128, KC, DH], BF16, tag="vw")

        for c in range(KC):
            nc.sync.dma_start(out=kw_sb[:, c, :],
                              in_=dd["kw_d"][128 * c:128 * (c + 1), :])
            nc.sync.dma_start(out=vw_sb[:, c, :],
                              in_=dd["vw_d"][128 * c:128 * (c + 1), :])
        nc.sync.dma_start(out=kb_sb, in_=dd["kb_d"])
        nc.sync.dma_start(out=qb_sb, in_=dd["qb_d"])
        nc.sync.dma_start(out=vbr_sb, in_=dd["vbr_d"])
        for m in range(VMC):
            nc.sync.dma_start(out=blb_sb[:, m, :],
                              in_=dd["blb_d"][128 * m:128 * (m + 1), :])
            nc.sync.dma_start(out=pw_sb[:, m, :],
                              in_=dd["pw_d"][128 * m:128 * (m + 1), :])
        nc.sync.dma_start(out=bt_sb[0:KEY0, 0, :], in_=dd["bt_d"][0:KEY0, :])
        nc.sync.dma_start(out=bt_sb[0:KEY1, 1, :], in_=dd["bt_d"][KEY0:N, :])
        nc.sync.dma_start(out=pb_sb, in_=dd["pb_d"])
        nc.sync.dma_start(out=id_sb, in_=dd["id_d"])
        nc.sync.dma_start(out=on_sb, in_=dd["on_d"])

        with tc.tile_pool(name="persist", bufs=1) as pp:
            k_s = pp.tile([128, e, N], BF16, tag="k_s")
            q_s = pp.tile([128, e, N2], BF16, tag="q_s")
            vt0 = pp.tile([128, e, DH], BF16, tag="vt0")
            vt1 = pp.tile([128, e, DH], BF16, tag="vt1")
            vl_sb = pp.tile([128, VMC, e, N2], BF16, tag="vl_sb")

            if parts & 1:
                phase1a(nc, tc, e, ndw, dd, qb_sb, blb_sb, q_s, vl_sb)
            if parts & 2:
                phase1b(nc, tc, e, ndw, dd, kw_sb, kb_sb, vw_sb, vbr_sb,
                        k_s, vt0, vt1)
            if parts & 4:
                phase2(nc, tc, e, ngrp, dd, id_sb, bt_sb, on_sb, pw_sb,
                       pb_sb, k_s, q_s, vt0, vt1, vl_sb, parts)


def phase1a(nc, tc, e, ndw, dd, qb_sb, blb_sb, q_s, vl_sb):
    """Fused depthwise taps: q (lgquery+pool+proj) and v_local, via im2col."""
    xcol_r = dd["xcol_d"].rearrange("(c p) (t ee q) -> p c t ee q",
                                    p=128, t=9, q=N2)
    with (tc.tile_pool(name="p1a", bufs=1) as p1a,
          tc.tile_pool(name="p1ax", bufs=2) as p1ax,
          tc.tile_pool(name="p1ap", bufs=2, space="PSUM") as ps1a):
        qt_sb = p1a.tile([128, KC, 9, 128], BF16, tag="qt")
        wt_sb = p1a.tile([128, KC, 9, DH], BF16, tag="wt")
        for c in range(KC):
            nc.sync.dma_start(out=qt_sb[:, c, :, :],
                              in_=dd["qt_d"][:, 128 * c:128 * (c + 1), :]
                              .rearrange("t p m -> p t m"))
            nc.sync.dma_start(out=wt_sb[:, c, :, :],
                              in_=dd["wt_d"][:, 128 * c:128 * (c + 1), :]
                              .rearrange("t p m -> p t m"))
        for blk in range(ndw):
            e0 = blk * NEG
            xc = p1ax.tile([128, KC, 9, NEG, N2], BF16, tag="xc")
            for c in range(KC):
                nc.sync.dma_start(out=xc[:, c, :, :, :],
                                  in_=xcol_r[:, c, :, e0:e0 + NEG, :])
            qp = ps1a.tile([128, NEG * N2], F32, tag="dwq")
            first = True
            for t in range(9):
                for c in range(KC):
                    nc.tensor.matmul(qp, qt_sb[:, c, t, :], xc[:, c, t, :, :],
                                     start=first, stop=(t == 8 and c == KC - 1))
                    first = False
            nc.scalar.activation(
                out=q_s[:, e0:e0 + NEG, :],
                in_=qp.rearrange("p (ee q) -> p ee q", q=N2),
                func=AF.Identity, bias=qb_sb)
            for m in range(VMC):
                vp = ps1a.tile([128, NEG * N2], F32, tag="dwq")
                first = True
                for t in range(9):
                    for c in range(KC):
                        nc.tensor.matmul(
                            vp, wt_sb[:, c, t, 128 * m:128 * (m + 1)],
                            xc[:, c, t, :, :],
                            start=first, stop=(t == 8 and c == KC - 1))
                        first = False
                nc.vector.tensor_add(
                    vl_sb[:, m, e0:e0 + NEG, :],
                    vp.rearrange("p (ee q) -> p ee q", q=N2),
                    blb_sb[:, m, :].rearrange("p (ee q) -> p ee q", q=N2))


def phase1b(nc, tc, e, ndw, dd, kw_sb, kb_sb, vw_sb, vbr_sb, k_s, vt0, vt1):
    """k (natural) and vT (keys on partitions) from flat x."""
    xfl_r = dd["xfl_d"].rearrange("(c p) (ee q) -> p c ee q", p=128, q=N)
    with (tc.tile_pool(name="p1bx", bufs=2) as p1bx,
          tc.tile_pool(name="p1bp", bufs=2, space="PSUM") as ps1b):
        for blk in range(ndw):
            e0 = blk * NEG
            xf = p1bx.tile([128, KC, NEG, N], BF16, tag="xf")
            for c in range(KC):
                nc.sync.dma_start(out=xf[:, c, :, :],
                                  in_=xfl_r[:, c, e0:e0 + NEG, :])
            for ee in range(NEG):
                ex = e0 + ee
                kp = ps1b.tile([128, N], F32, tag="kp")
                for c in range(KC):
                    nc.tensor.matmul(kp, kw_sb[:, c, :], xf[:, c, ee, :],
                                     start=(c == 0), stop=(c == KC - 1))
                nc.scalar.activation(out=k_s[:, ex, :], in_=kp,
                                     func=AF.Identity, bias=kb_sb)
                v0 = ps1b.tile([126, DH], F32, tag="vt0p")
                v1 = ps1b.tile([70, DH], F32, tag="vt1p")
                for c in range(KC):
                    nc.tensor.matmul(v0, xf[:, c, ee, 0:KEY0], vw_sb[:, c, :],
                                     start=(c == 0), stop=(c == KC - 1))
                for c in range(KC):
                    nc.tensor.matmul(v1, xf[:, c, ee, KEY0:N], vw_sb[:, c, :],
                                     start=(c == 0), stop=(c == KC - 1))
                nc.vector.tensor_add(vt0[0:126, ex, :], v0, vbr_sb[0:126, :])
                nc.vector.tensor_add(vt1[0:70, ex, :], v1, vbr_sb[0:70, :])


def phase2(nc, tc, e, ngrp, dd, id_sb, bt_sb, on_sb, pw_sb, pb_sb,
           k_s, q_s, vt0, vt1, vl_sb, parts=15):
    """Attention (transposed layout) then gelu + proj + store."""
    with (tc.tile_pool(name="p2s", bufs=1) as p2,
          tc.tile_pool(name="p2d", bufs=2) as p2d):
        qbd = p2.tile([128, e, HQ], BF16, tag="qbd")
        g_sb = p2.tile([128, VMC, e, N2], BF16, tag="g_sb")

        # blockdiag q: zero once, scatter blocks via sbuf->sbuf DMA
        nc.gpsimd.memset(qbd, 0)
        for h in range(H):
            nc.sync.dma_start(
                out=qbd[16 * h:16 * (h + 1), :, N2 * h:N2 * (h + 1)],
                in_=q_s[16 * h:16 * (h + 1), :, :])

        with tc.tile_pool(name="p2p", bufs=2, space="PSUM") as ps2:
            for grp in range(ngrp):
                a0 = p2d.tile([128, GEX, HQ], BF16, tag="att0")
                a1 = p2d.tile([128, GEX, HQ], BF16, tag="att1")
                rsf = p2d.tile([128, GEX, HQ], F32, tag="rsf")
                rsb = p2d.tile([128, GEX, HQ], BF16, tag="rsb")
                for eg in range(GEX):
                    ex = grp * GEX + eg
                    l0 = ps2.tile([126, HQ], F32, tag="l0")
                    l1 = ps2.tile([70, HQ], F32, tag="l1")
                    # transposed logits + attention bias
                    nc.tensor.matmul(l0, k_s[:, ex, 0:KEY0], qbd[:, ex, :],
                                     start=True, stop=False)
                    nc.tensor.matmul(l0, id_sb[0:126, 0:126],
                                     bt_sb[0:126, 0, :], start=False, stop=True)
                    nc.tensor.matmul(l1, k_s[:, ex, KEY0:N], qbd[:, ex, :],
                                     start=True, stop=False)
                    nc.tensor.matmul(l1, id_sb[0:70, 0:70], bt_sb[0:70, 1, :],
                                     start=False, stop=True)
                    nc.scalar.activation(out=a0[0:126, eg, :], in_=l0,
                                         func=AF.Exp)
                    nc.scalar.activation(out=a1[0:70, eg, :], in_=l1,
                                         func=AF.Exp)
                    # denominators, replicated across partitions
                    sp = ps2.tile([128, HQ], F32, tag="sp")
                    nc.tensor.matmul(sp, on_sb[0:126, :], a0[0:126, eg, :],
                                     start=True, stop=False)
                    nc.tensor.matmul(sp, on_sb[0:70, :], a1[0:70, eg, :],
                                     start=False, stop=True)
                    nc.vector.reciprocal_approx_fast(out=rsf[:, eg, :], in_=sp)
                    nc.vector.tensor_copy(rsb[:, eg, :], rsf[:, eg, :])
                    nc.vector.tensor_mul(a0[0:126, eg, :], a0[0:126, eg, :],
                                         rsb[0:126, eg, :])
                    nc.vector.tensor_mul(a1[0:70, eg, :], a1[0:70, eg, :],
                                         rsb[0:70, eg, :])
                # AV: head pairs fused into M=128 matmuls (adjacent att
                # columns act as the block-diagonal rhs; off-head quadrants
                # are garbage, never read).  One accumulation group per bank.
                for eg in range(GEX):
                    ex = grp * GEX + eg
                    oe = ps2.tile([128, VMC, 2, N2], F32, tag="oeg")
                    mms = []
                    for m in range(VMC):
                        for kc in range(2):
                            vt, aa, kk = ((vt0, a0, 126) if kc == 0
                                          else (vt1, a1, 70))
                            i = len(mms)
                            mm = nc.tensor.matmul(
                                oe[:, m, :, :],
                                vt[0:kk, ex, 128 * m:128 * (m + 1)],
                                aa[0:kk, eg, 98 * m:98 * (m + 1)],
                                start=(i == 0), stop=(i == 2 * VMC - 1))
                            mms.append(mm)
                    for mm in mms[1:-1]:
                        tile.add_dep_helper(mm.ins, mms[0].ins, sync=False,
                                            reason="psum group start first")
                        tile.add_dep_helper(mms[-1].ins, mm.ins, sync=False,
                                            reason="psum group stop last")
                    tile.add_dep_helper(mms[-1].ins, mms[0].ins, sync=False,
                                        reason="psum group order")
                    # merge with v_local -> gelu input, per partition half
                    nc.vector.tensor_add(g_sb[0:64, :, ex, :],
                                         oe[0:64, :, 0, :],
                                         vl_sb[0:64, :, ex, :])
                    nc.vector.tensor_add(g_sb[64:128, :, ex, :],
                                         oe[64:128, :, 1, :],
                                         vl_sb[64:128, :, ex, :])

        if not parts & 8:
            return
        # ---------------- phase 3: gelu + proj + store ----------------------
        with tc.tile_pool(name="p3p", bufs=4, space="PSUM") as ps3:
            for m in range(VMC):
                nc.scalar.activation(out=g_sb[:, m, :, :],
                                     in_=g_sb[:, m, :, :], func=AF.Gelu)
            for m in range(PMC):
                ost = p2d.tile([128, e, N2], F32, tag="ost")
                for grp in range(ngrp):
                    e0 = grp * GEX
                    pj = ps3.tile([128, GEX, N2], F32, tag="pj")
                    for c in range(VMC):
                        nc.tensor.matmul(pj,
                                         pw_sb[:, c, 128 * m:128 * (m + 1)],
                                         g_sb[:, c, e0:e0 + GEX, :],
                                         start=(c == 0), stop=(c == VMC - 1))
                    nc.scalar.activation(out=ost[:, e0:e0 + GEX, :], in_=pj,
                                         func=AF.Identity,
                                         bias=pb_sb[:, m:m + 1])
                nc.sync.dma_start(
                    out=dd["out_d"][:, 128 * m:128 * (m + 1), :]
                    .rearrange("e p q -> p e q"),
                    in_=ost)


_CACHE = {}


def _get_nc(e=E, loops=0, parts=15):
    key = (e, loops, parts)
    if key not in _CACHE:
        _CACHE[key] = build_nc(e, loops, parts)
    return _CACHE[key]


def kernel(**inputs):
    from concourse.bass_utils import run_bass_kernel_spmd

    shared, x_shards = host_prep(inputs)
    nc = _get_nc(E)
    in_maps = [{**shared, **x_shards[c]} for c in range(NCORES)]
    res = run_bass_kernel_spmd(nc, in_maps, core_ids=list(range(NCORES)))
    out = np.concatenate([r["out"] for r in res.results], axis=0)
    return out.reshape(B, OUT_DIM, RES2, RES2).astype(np.float32)


# revision 21
# speedup vs baseline: 2.2331x; 2.2331x over previous
"""nn_Attention4DDownsample — Trainium2 Bass/Tile kernel.

Sharding: pure data parallel over batch (8 cores x 32 examples), weights
replicated.  All matmuls bf16.  Host pre-folds BN into conv weights, fuses
the depthwise 3x3 s2 convs with their neighbouring 1x1 convs into 9 per-tap
dense matrices (pool folded into the centre tap), builds an im2col copy of
x for the taps, and pre-gathers the attention bias table.

Attention is computed transposed (keys on partitions):
  logitsT[key,(h,q)] = contraction of k (natural layout) with blockdiag(q)
so softmax exp/sums and the AV matmul need no on-chip transposes at all.
The softmax denominator is produced replicated across partitions via an
all-ones matmul, so normalisation is a plain elementwise multiply; AV fuses
head pairs into M=128 matmuls whose off-head quadrants are never read.
"""

import contextlib
import os
import sys

import numpy as np

for _p in ("/opt/trn_rl_repo", "/root/.axon_site/_ro/trn_rl_repo"):
    if _p not in sys.path and os.path.isdir(_p):
        sys.path.append(_p)

import ml_dtypes  # noqa: E402

import concourse.bacc as bacc  # noqa: E402
import concourse.mybir as mybir  # noqa: E402
import concourse.tile as tile  # noqa: E402

BF16 = mybir.dt.bfloat16
F32 = mybir.dt.float32
AF = mybir.ActivationFunctionType

B, DIM, RES = 256, 384, 14
H, KD, D = 8, 16, 64
NH_KD, DH = H * KD, H * D  # 128, 512
OUT_DIM = 768
RES2 = 7
N, N2 = RES * RES, RES2 * RES2  # 196, 49
SCALE = KD ** -0.5
NCORES = 8
E = B // NCORES  # 32 examples per core

KC = DIM // 128        # 3 contraction chunks of x-channels
VMC = DH // 128        # 4 m-chunks of v channels
PMC = OUT_DIM // 128   # 6 m-chunks of proj out channels
HQ = H * N2            # 392
# key split along the 14x14 grid: first 9 rows (126 keys) + last 5 (70 keys)
KEY0, KEY1 = 9 * RES, 5 * RES  # 126, 70
PAD = RES + 1  # 15 (top/left zero pad only)

NEG = 8   # examples per dwconv N-chunk (N = 8*49 = 392)
GEX = 4   # examples per attention group


def _bf(x):
    return np.ascontiguousarray(np.asarray(x, np.float32).astype(ml_dtypes.bfloat16))


def _f32(x):
    return np.ascontiguousarray(np.asarray(x, np.float32))


def host_prep(inputs):
    """Fold BN, fuse dwconvs, build im2col, gather bias."""
    ii = {k: (np.asarray(v) if np.asarray(v).dtype == np.int32
              else np.asarray(v, np.float32)) for k, v in inputs.items()}

    kw = ii["k_w"][:, :, 0, 0] * ii["k_bn_s"][:, None]          # [128,384]
    kb = ii["k_b"] * ii["k_bn_s"] + ii["k_bn_b"]                # [128]
    qw = SCALE * ii["q_bn_s"][:, None] * ii["q_proj_w"][:, :, 0, 0]   # [128,384]
    qlw = ii["q_local_w"][:, 0].reshape(DIM, 9).copy()                # [384,9]
    qlw[:, 4] += 1.0                                                  # pool
    q_taps = np.einsum("md,dt->tdm", qw, qlw)                         # [9,384,128]
    qb = (SCALE * (ii["q_bn_s"] * ii["q_proj_b"] + ii["q_bn_b"])
          + qw @ ii["q_local_b"])                                     # [128]
    vw = ii["v_w"][:, :, 0, 0] * ii["v_bn_s"][:, None]          # [512,384]
    vb = ii["v_b"] * ii["v_bn_s"] + ii["v_bn_b"]                # [512]
    vlw = ii["vl_w"][:, 0].reshape(DH, 9) * ii["vl_bn_s"][:, None]  # [512,9]
    vlb = ii["vl_b"] * ii["vl_bn_s"] + ii["vl_bn_b"]            # [512]
    w_taps = np.einsum("cd,ct->tdc", vw, vlw)                   # [9,384,512]
    # dwconv bias: vb contributes only where a tap hits the interior
    interior = np.zeros((9, N2), np.float32)
    for t in range(9):
        di, dj = t // 3, t % 3
        for i in range(RES2):
            for j in range(RES2):
                r, c = 2 * i + di - 1, 2 * j + dj - 1
                if 0 <= r < RES and 0 <= c < RES:
                    interior[t, i * RES2 + j] = 1.0
    blb = vlb[:, None] + vb[:, None] * (vlw @ interior)         # [512,49]
    blb_rep = np.tile(blb[:, None, :], (1, NEG, 1))             # [512,NEG,49]
    bias = ii["attn_biases"][:, ii["bias_idxs"]]                # [8,49,196]
    biasT = np.transpose(bias, (2, 0, 1)).reshape(N, HQ)        # [196,392]
    pw = ii["proj_w"][:, :, 0, 0] * ii["proj_bn_s"][:, None]    # [768,512]
    pb = ii["proj_b"] * ii["proj_bn_s"] + ii["proj_bn_b"]       # [768]

    shared = {
        "kw_t": _bf(kw.T),                        # [384,128]
        "kb": _f32(kb)[:, None],                  # [128,1]
        "q_taps": _bf(q_taps),                    # [9,384,128]
        "qb": _f32(qb)[:, None],                  # [128,1]
        "w_taps": _bf(w_taps),                    # [9,384,512]
        "vw_t": _bf(vw.T),                        # [384,512]
        "vb_rep": _f32(np.tile(vb[None, :], (128, 1))),  # [128,512]
        "blb_rep": _bf(blb_rep.reshape(DH, NEG * N2)),   # [512,392]
        "biasT": _bf(biasT),                      # [196,392]
        "pw_t": _bf(pw.T),                        # [512,768]
        "pb": _f32(np.ascontiguousarray(pb.reshape(PMC, 128).T)),  # [128,6]
        "ident": _bf(np.eye(128)),                # [128,128]
        "ones_t": _bf(np.ones((128, 128))),       # [128,128]
    }

    # x: shard; flat channel-major copy + host im2col of the 9 dw taps
    x = ii["x"].reshape(NCORES, E, DIM, RES, RES)
    xp = np.zeros((NCORES, E, DIM, PAD, PAD), np.float32)
    xp[:, :, :, 1:, 1:] = x
    xp = np.transpose(xp, (0, 2, 1, 3, 4))        # [8,384,E,15,15]
    x_shards = []
    for c in range(NCORES):
        xflat = _bf(np.transpose(x[c], (1, 0, 2, 3)).reshape(DIM, E * N))
        taps = np.stack(
            [xp[c][:, :, t // 3:t // 3 + 13:2, t % 3:t % 3 + 13:2]
             .reshape(DIM, E, N2) for t in range(9)], axis=1)  # [384,9,E,49]
        x_shards.append({"xfl": xflat, "xcol": _bf(taps.reshape(DIM, 9 * E * N2))})
    return shared, x_shards


def build_nc(e=E, loops=0, parts=15):
    """Build the Bass program for one core with `e` examples.

    loops>0 wraps the body in a hardware For loop (benchmark builds).
    parts: bitmask 1=dwconv 2=k/vT 4=attention 8=gelu/proj (bench only).
    """
    ndw = e // NEG      # dwconv / stream blocks
    ngrp = e // GEX     # attention groups
    assert e % NEG == 0 and e % GEX == 0

    nc = bacc.Bacc("TRN2", target_bir_lowering=False, debug=False,
                   enable_asserts=False, num_devices=NCORES)

    def din(name, shape, dtype=BF16):
        return nc.dram_tensor(name, list(shape), dtype, kind="ExternalInput").ap()

    dd = {
        "xfl_d": din("xfl", (DIM, e * N)),
        "xcol_d": din("xcol", (DIM, 9 * e * N2)),
        "kw_d": din("kw_t", (DIM, 128)),
        "kb_d": din("kb", (128, 1), F32),
        "qt_d": din("q_taps", (9, DIM, 128)),
        "qb_d": din("qb", (128, 1), F32),
        "wt_d": din("w_taps", (9, DIM, DH)),
        "vw_d": din("vw_t", (DIM, DH)),
        "vbr_d": din("vb_rep", (128, DH), F32),
        "blb_d": din("blb_rep", (DH, NEG * N2)),
        "bt_d": din("biasT", (N, HQ)),
        "pw_d": din("pw_t", (DH, OUT_DIM)),
        "pb_d": din("pb", (128, PMC), F32),
        "id_d": din("ident", (128, 128)),
        "on_d": din("ones_t", (128, 128)),
        "out_d": nc.dram_tensor("out", [e, OUT_DIM, N2], F32,
                                kind="ExternalOutput").ap(),
    }

    with tile.TileContext(nc) as tc:
        with (tc.For_i(0, loops, 1) if loops else contextlib.nullcontext()):
            build_body(nc, tc, e, ndw, ngrp, dd, parts)

    nc.compile()
    return nc


def build_body(nc, tc, e, ndw, ngrp, dd, parts=15):
    with tc.tile_pool(name="const", bufs=1) as cp:
        kw_sb = cp.tile([128, KC, 128], BF16, tag="kw")
        kb_sb = cp.tile([128, 1], F32, tag="kb")
        qb_sb = cp.tile([128, 1], F32, tag="qb")
        vbr_sb = cp.tile([128, DH], F32, tag="vbr")
        blb_sb = cp.tile([128, VMC, NEG * N2], BF16, tag="blb")
        bt_sb = cp.tile([128, 2, HQ], BF16, tag="bt")
        pw_sb = cp.tile([128, VMC, OUT_DIM], BF16, tag="pw")
        pb_sb = cp.tile([128, PMC], F32, tag="pb")
        id_sb = cp.tile([128, 128], BF16, tag="id")
        on_sb = cp.tile([128, 128], BF16, tag="on")
        vw_sb = cp.tile([128, KC, DH], BF16, tag="vw")
        vbb_sb = cp.tile([128, DH], BF16, tag="vbb")

        for c in range(KC):
            nc.sync.dma_start(out=kw_sb[:, c, :],
                              in_=dd["kw_d"][128 * c:128 * (c + 1), :])
            nc.sync.dma_start(out=vw_sb[:, c, :],
                              in_=dd["vw_d"][128 * c:128 * (c + 1), :])
        nc.sync.dma_start(out=kb_sb, in_=dd["kb_d"])
        nc.sync.dma_start(out=qb_sb, in_=dd["qb_d"])
        nc.sync.dma_start(out=vbr_sb, in_=dd["vbr_d"])
        nc.vector.tensor_copy(vbb_sb, vbr_sb)
        for m in range(VMC):
            nc.sync.dma_start(out=blb_sb[:, m, :],
                              in_=dd["blb_d"][128 * m:128 * (m + 1), :])
            nc.sync.dma_start(out=pw_sb[:, m, :],
                              in_=dd["pw_d"][128 * m:128 * (m + 1), :])
        nc.sync.dma_start(out=bt_sb[0:KEY0, 0, :], in_=dd["bt_d"][0:KEY0, :])
        nc.sync.dma_start(out=bt_sb[0:KEY1, 1, :], in_=dd["bt_d"][KEY0:N, :])
        nc.sync.dma_start(out=pb_sb, in_=dd["pb_d"])
        nc.sync.dma_start(out=id_sb, in_=dd["id_d"])
        nc.sync.dma_start(out=on_sb, in_=dd["on_d"])

        with tc.tile_pool(name="persist", bufs=1) as pp:
            k_s = pp.tile([128, e, N], BF16, tag="k_s")
            q_s = pp.tile([128, e, N2], BF16, tag="q_s")
            vt0 = pp.tile([128, e, DH], BF16, tag="vt0")
            vt1 = pp.tile([128, e, DH], BF16, tag="vt1")
            vl_sb = pp.tile([128, VMC, e, N2], BF16, tag="vl_sb")

            if parts & 1:
                phase1a(nc, tc, e, ndw, dd, qb_sb, blb_sb, q_s, vl_sb)
            if parts & 2:
                phase1b(nc, tc, e, ndw, dd, kw_sb, kb_sb, vw_sb, vbr_sb,
                        vbb_sb, on_sb, k_s, vt0, vt1)
            if parts & 4:
                phase2(nc, tc, e, ngrp, dd, id_sb, bt_sb, on_sb, pw_sb,
                       pb_sb, k_s, q_s, vt0, vt1, vl_sb, parts)


def phase1a(nc, tc, e, ndw, dd, qb_sb, blb_sb, q_s, vl_sb):
    """Fused depthwise taps: q (lgquery+pool+proj) and v_local, via im2col."""
    xcol_r = dd["xcol_d"].rearrange("(c p) (t ee q) -> p c t ee q",
                                    p=128, t=9, q=N2)
    with (tc.tile_pool(name="p1a", bufs=1) as p1a,
          tc.tile_pool(name="p1ax", bufs=2) as p1ax,
          tc.tile_pool(name="p1ap", bufs=2, space="PSUM") as ps1a):
        qt_sb = p1a.tile([128, KC, 9, 128], BF16, tag="qt")
        wt_sb = p1a.tile([128, KC, 9, DH], BF16, tag="wt")
        for c in range(KC):
            nc.sync.dma_start(out=qt_sb[:, c, :, :],
                              in_=dd["qt_d"][:, 128 * c:128 * (c + 1), :]
                              .rearrange("t p m -> p t m"))
            nc.sync.dma_start(out=wt_sb[:, c, :, :],
                              in_=dd["wt_d"][:, 128 * c:128 * (c + 1), :]
                              .rearrange("t p m -> p t m"))
        for blk in range(ndw):
            e0 = blk * NEG
            xc = p1ax.tile([128, KC, 9, NEG, N2], BF16, tag="xc")
            for c in range(KC):
                nc.sync.dma_start(out=xc[:, c, :, :, :],
                                  in_=xcol_r[:, c, :, e0:e0 + NEG, :])
            qp = ps1a.tile([128, NEG * N2], F32, tag="dwq")
            first = True
            for t in range(9):
                for c in range(KC):
                    nc.tensor.matmul(qp, qt_sb[:, c, t, :], xc[:, c, t, :, :],
                                     start=first, stop=(t == 8 and c == KC - 1))
                    first = False
            nc.scalar.activation(
                out=q_s[:, e0:e0 + NEG, :],
                in_=qp.rearrange("p (ee q) -> p ee q", q=N2),
                func=AF.Identity, bias=qb_sb)
            for m in range(VMC):
                vp = ps1a.tile([128, NEG * N2], F32, tag="dwq")
                first = True
                for t in range(9):
                    for c in range(KC):
                        nc.tensor.matmul(
                            vp, wt_sb[:, c, t, 128 * m:128 * (m + 1)],
                            xc[:, c, t, :, :],
                            start=first, stop=(t == 8 and c == KC - 1))
                        first = False
                nc.vector.tensor_add(
                    vl_sb[:, m, e0:e0 + NEG, :],
                    vp.rearrange("p (ee q) -> p ee q", q=N2),
                    blb_sb[:, m, :].rearrange("p (ee q) -> p ee q", q=N2))


def phase1b(nc, tc, e, ndw, dd, kw_sb, kb_sb, vw_sb, vbr_sb, vbb_sb, on_sb,
            k_s, vt0, vt1):
    """k (natural) and vT (keys on partitions) from flat x."""
    xfl_r = dd["xfl_d"].rearrange("(c p) (ee q) -> p c ee q", p=128, q=N)
    with (tc.tile_pool(name="p1bx", bufs=2) as p1bx,
          tc.tile_pool(name="p1bp", bufs=2, space="PSUM") as ps1b):
        for blk in range(ndw):
            e0 = blk * NEG
            xf = p1bx.tile([128, KC, NEG, N], BF16, tag="xf")
            for c in range(KC):
                nc.sync.dma_start(out=xf[:, c, :, :],
                                  in_=xfl_r[:, c, e0:e0 + NEG, :])
            for ee in range(0, NEG, 2):
                ex = e0 + ee
                kp = ps1b.tile([128, 2, N], F32, tag="kp")
                for c in range(KC):
                    nc.tensor.matmul(kp, kw_sb[:, c, :],
                                     xf[:, c, ee:ee + 2, :],
                                     start=(c == 0), stop=(c == KC - 1))
                nc.scalar.activation(out=k_s[:, ex:ex + 2, :], in_=kp,
                                     func=AF.Identity, bias=kb_sb)
            for ee in range(NEG):
                ex = e0 + ee
                v0 = ps1b.tile([126, DH], F32, tag="vt0p")
                v1 = ps1b.tile([70, DH], F32, tag="vt1p")
                for c in range(KC):
                    nc.tensor.matmul(v0, xf[:, c, ee, 0:KEY0], vw_sb[:, c, :],
                                     start=(c == 0), stop=(c == KC - 1))
                for c in range(KC):
                    nc.tensor.matmul(v1, xf[:, c, ee, KEY0:N], vw_sb[:, c, :],
                                     start=(c == 0), stop=(c == KC - 1))
                # vb bias: DVE add for the 126-chunk; K=1 ones-row matmul +
                # ACT copy for the 70-chunk (keeps DVE/ACT balanced)
                nc.vector.tensor_add(vt0[0:126, ex, :], v0, vbr_sb[0:126, :])
                nc.tensor.matmul(v1, on_sb[0:1, 0:70], vbb_sb[0:1, :],
                                 start=False, stop=True, skip_group_check=True)
                nc.scalar.activation(out=vt1[0:70, ex, :], in_=v1,
                                     func=AF.Identity)


def phase2(nc, tc, e, ngrp, dd, id_sb, bt_sb, on_sb, pw_sb, pb_sb,
           k_s, q_s, vt0, vt1, vl_sb, parts=15):
    """Attention (transposed layout) then gelu + proj + store."""
    with (tc.tile_pool(name="p2s", bufs=1) as p2,
          tc.tile_pool(name="p2d", bufs=2) as p2d):
        qbd = p2.tile([128, e, HQ], BF16, tag="qbd")
        g_sb = p2.tile([128, VMC, e, N2], BF16, tag="g_sb")

        # blockdiag q: zero once, scatter blocks via sbuf->sbuf DMA
        nc.gpsimd.memset(qbd, 0)
        for h in range(H):
            nc.sync.dma_start(
                out=qbd[16 * h:16 * (h + 1), :, N2 * h:N2 * (h + 1)],
                in_=q_s[16 * h:16 * (h + 1), :, :])

        with tc.tile_pool(name="p2p", bufs=2, space="PSUM") as ps2:
            for grp in range(ngrp):
                a0 = p2d.tile([128, GEX, HQ], BF16, tag="att0")
                a1 = p2d.tile([128, GEX, HQ], BF16, tag="att1")
                rsf = p2d.tile([128, GEX, HQ], F32, tag="rsf")
                rsb = p2d.tile([128, GEX, HQ], BF16, tag="rsb")
                for eg in range(GEX):
                    ex = grp * GEX + eg
                    l0 = ps2.tile([126, HQ], F32, tag="l0")
                    l1 = ps2.tile([70, HQ], F32, tag="l1")
                    # transposed logits + attention bias
                    nc.tensor.matmul(l0, k_s[:, ex, 0:KEY0], qbd[:, ex, :],
                                     start=True, stop=False)
                    nc.tensor.matmul(l0, id_sb[0:126, 0:126],
                                     bt_sb[0:126, 0, :], start=False, stop=True)
                    nc.tensor.matmul(l1, k_s[:, ex, KEY0:N], qbd[:, ex, :],
                                     start=True, stop=False)
                    nc.tensor.matmul(l1, id_sb[0:70, 0:70], bt_sb[0:70, 1, :],
                                     start=False, stop=True)
                    nc.scalar.activation(out=a0[0:126, eg, :], in_=l0,
                                         func=AF.Exp)
                    nc.scalar.activation(out=a1[0:70, eg, :], in_=l1,
                                         func=AF.Exp)
                    # denominators, replicated across partitions
                    sp = ps2.tile([128, HQ], F32, tag="sp")
                    nc.tensor.matmul(sp, on_sb[0:126, :], a0[0:126, eg, :],
                                     start=True, stop=False)
                    nc.tensor.matmul(sp, on_sb[0:70, :], a1[0:70, eg, :],
                                     start=False, stop=True)
                    nc.vector.reciprocal_approx_fast(out=rsf[:, eg, :], in_=sp)
                    nc.vector.tensor_copy(rsb[:, eg, :], rsf[:, eg, :])
                    nc.vector.tensor_mul(a0[0:126, eg, :], a0[0:126, eg, :],
                                         rsb[0:126, eg, :])
                    nc.vector.tensor_mul(a1[0:70, eg, :], a1[0:70, eg, :],
                                         rsb[0:70, eg, :])
                # AV: head pairs fused into M=128 matmuls (adjacent att
                # columns act as the block-diagonal rhs; off-head quadrants
                # are garbage, never read).  One accumulation group per bank.
                for eg in range(GEX):
                    ex = grp * GEX + eg
                    oe = ps2.tile([128, VMC, 2, N2], F32, tag="oeg")
                    mms = []
                    for m in range(VMC):
                        for kc in range(2):
                            vt, aa, kk = ((vt0, a0, 126) if kc == 0
                                          else (vt1, a1, 70))
                            i = len(mms)
                            mm = nc.tensor.matmul(
                                oe[:, m, :, :],
                                vt[0:kk, ex, 128 * m:128 * (m + 1)],
                                aa[0:kk, eg, 98 * m:98 * (m + 1)],
                                start=(i == 0), stop=(i == 2 * VMC - 1))
                            mms.append(mm)
                    for mm in mms[1:-1]:
                        tile.add_dep_helper(mm.ins, mms[0].ins, sync=False,
                                            reason="psum group start first")
                        tile.add_dep_helper(mms[-1].ins, mm.ins, sync=False,
                                            reason="psum group stop last")
                    tile.add_dep_helper(mms[-1].ins, mms[0].ins, sync=False,
                                        reason="psum group order")
                    # merge with v_local -> gelu input, per partition half
                    nc.vector.tensor_add(g_sb[0:64, :, ex, :],
                                         oe[0:64, :, 0, :],
                                         vl_sb[0:64, :, ex, :])
                    nc.vector.tensor_add(g_sb[64:128, :, ex, :],
                                         oe[64:128, :, 1, :],
                                         vl_sb[64:128, :, ex, :])

        if not parts & 8:
            return
        # ---------------- phase 3: gelu + proj + store ----------------------
        with tc.tile_pool(name="p3p", bufs=4, space="PSUM") as ps3:
            for m in range(VMC):
                nc.scalar.activation(out=g_sb[:, m, :, :],
                                     in_=g_sb[:, m, :, :], func=AF.Gelu)
            for m in range(PMC):
                ost = p2d.tile([128, e, N2], F32, tag="ost")
                for e0 in range(0, e, NEG):
                    pj = ps3.tile([128, NEG, N2], F32, tag="pj")
                    for c in range(VMC):
                        nc.tensor.matmul(pj,
                                         pw_sb[:, c, 128 * m:128 * (m + 1)],
                                         g_sb[:, c, e0:e0 + NEG, :],
                                         start=(c == 0), stop=(c == VMC - 1))
                    nc.scalar.activation(out=ost[:, e0:e0 + NEG, :], in_=pj,
                                         func=AF.Identity,
                                         bias=pb_sb[:, m:m + 1])
                nc.sync.dma_start(
                    out=dd["out_d"][:, 128 * m:128 * (m + 1), :]
                    .rearrange("e p q -> p e q"),
                    in_=ost)


_CACHE = {}


def _get_nc(e=E, loops=0, parts=15):
    key = (e, loops, parts)
    if key not in _CACHE:
        _CACHE[key] = build_nc(e, loops, parts)
    return _CACHE[key]


def kernel(**inputs):
    from concourse.bass_utils import run_bass_kernel_spmd

    shared, x_shards = host_prep(inputs)
    nc = _get_nc(E)
    in_maps = [{**shared, **x_shards[c]} for c in range(NCORES)]
    res = run_bass_kernel_spmd(nc, in_maps, core_ids=list(range(NCORES)))
    out = np.concatenate([r["out"] for r in res.results], axis=0)
    return out.reshape(B, OUT_DIM, RES2, RES2).astype(np.float32)
